# revision 7
# baseline (speedup 1.0000x reference)
"""Trainium2 Bass kernel for nn_Decoder_25718264168590 (v4).

2-layer LSTM decoder (B=32, T=50, H=1024, E=128) + vocab projection
(V=32000) + log_softmax, on 8 NeuronCores.

This environment executes instructions at ~60-95 us each (flat in data
size), so the design minimizes instruction count on the PE chain and
collective/barrier count:

- Gate-sharded recurrence: core r owns h-units [128r, 128r+128) of both
  layers.  Layer 1 lags layer 0 by one tick; both layers' gates are
  computed per tick as two 512-col PSUM groups: g0 (8 k-tile matmuls
  over h0) and g1 (16 k-tiles over [h0; h1]).  24 matmuls/tick.
- The x-path (target @ A1) + enc-path + biases are folded into a
  26-matmul PRE-PASS producing a SBUF-resident gxb table [32, T*512];
  per tick the bias/x contribution is added by one DVE op reading PSUM.
- ONE AllGather per tick ships [h0(tau)^T | h1(tau-2)^T] = [128, 64]
  bf16 (edge ticks ship [128, 32]).  52 barriers total vs 104, and the
  L1 lag of TWO ticks means the wih1 matmuls depend only on 2-tick-old
  AG data: per tick the PE chain runs [vocab chunks | wih1] (no wait)
  before [whh1 | g0] (1-tick-old AG), hiding most of the AG latency.
- The lse AllReduce is split: the reduce fires 2 ticks before the
  output pass consumes lse, so the AR latency never head-blocks the
  DVE queue.
- Both cells' nonlinearities are fused: per-core gate columns are
  arranged [i0 i1 f0 f1 o0 o1 g0 g1] (128 each), so one sigmoid over
  [32, 768]-strided, one tanh over [32, 256], one tanh over c [32,256],
  and 4 wide DVE ops update both layers at once.
- Vocab projection: vocab-sharded (4000 cols/core), fp8e4 DoubleRow,
  spread as 2-chunk pairs into EVERY tick (scheduled >=2 ticks after
  their h1 slices land so they never wait on the newest AG); chunked
  AllReduce for the log-sum-exp; outputs written as log-softmax.
"""

import sys

for _p in ("/opt/trn_rl_repo",):
    if _p not in sys.path:
        sys.path.insert(0, _p)

import numpy as np
import ml_dtypes

B, T, H, E, V = 32, 50, 1024, 128, 32000
NCORES = 8
VS = V // NCORES          # 4000 vocab cols per core
S = B * T                 # 1600 samples, t-major on device: s = t*32 + b
KT = H // 128             # 8 k-tiles of hidden per layer
NMT = 13                  # sample m-tiles in vocab phase (12*128 + 64)
NCHK = 8                  # vocab col chunks per core (8 * 500)
CHUNK = VS // NCHK        # 500
AR_CHUNKS = ((0, 4), (4, 8), (8, 12), (12, 13))

SW = 2048.0               # fp8 weight scale (|w|<=0.1 -> <=204.8)
SH = 128.0                # fp8 h1 scale (|h|<=1 -> <=128)
DESCALE = 1.0 / (SW * SH)

BF16 = ml_dtypes.bfloat16
FP8 = ml_dtypes.float8_e4m3

_BUILD_CACHE = {}

# gate-column arrangement inside each core's 1024 cols:
# [i0 i1 f0 f1 o0 o1 g0 g1], 128 each.  PSUM g0 holds L0's (i0 f0 o0 g0)
# contiguously; PSUM g1 holds L1's (i1 f1 o1 g1).  The DVE add that
# moves PSUM->SBUF scatters into the interleaved arrangement.
# torch gate order in weights: i, f, g, o.
G_L0 = (0, 1, 3, 2)       # torch (i,f,g,o) -> psum order (i, f, o, g)
G_L1 = (0, 1, 3, 2)


def _host_prep(inputs):
    enc = np.asarray(inputs["enc_output"], np.float32)       # (B, H)
    target = np.asarray(inputs["target"], np.float32)        # (B, T, E)
    proj_w = np.asarray(inputs["proj_w"], np.float32)        # (E, H+E)
    proj_b = np.asarray(inputs["proj_b"], np.float32)        # (E,)
    w_ih0 = np.asarray(inputs["w_ih0"], np.float32)          # (4H, E)
    w_hh0 = np.asarray(inputs["w_hh0"], np.float32)          # (4H, H)
    b0 = np.asarray(inputs["b_ih0"], np.float32) + np.asarray(inputs["b_hh0"], np.float32)
    w_ih1 = np.asarray(inputs["w_ih1"], np.float32)          # (4H, H)
    w_hh1 = np.asarray(inputs["w_hh1"], np.float32)          # (4H, H)
    b1 = np.asarray(inputs["b_ih1"], np.float32) + np.asarray(inputs["b_hh1"], np.float32)
    lin_w = np.asarray(inputs["lin_w"], np.float32)          # (V, H)
    lin_b = np.asarray(inputs["lin_b"], np.float32)          # (V,)

    P1 = proj_w[:, :E].T                                     # (E, E)
    P2 = proj_w[:, E:].T                                     # (H, E)
    A1 = P1 @ w_ih0.T                                        # (E, 4H) x-path fold
    genc = (enc @ P2 + proj_b) @ w_ih0.T + b0                # (B, 4H) enc fold + b0

    # t-major input features: xt[e, t*32+b] = target[b, t, e]
    xt = np.ascontiguousarray(
        target.transpose(1, 0, 2).reshape(S, E).T).astype(BF16)   # (128, 1600)

    # exchange-layout encoder init: ench[p, k, b] = enc[b, k*128+p]
    ench = np.ascontiguousarray(
        enc.T.reshape(KT, 128, B).transpose(1, 0, 2))             # (128, KT, B)

    lin_wT = lin_w.T                                         # (H, V)
    use_linb = bool(np.any(lin_b != 0.0))

    in_maps = []
    for r in range(NCORES):
        u = r * 128                                          # h-unit base
        rows0 = np.concatenate(
            [np.arange(128) + g * H + u for g in G_L0])      # L0's 512 gate rows
        rows1 = np.concatenate(
            [np.arange(128) + g * H + u for g in G_L1])      # L1's 512 gate rows
        m = {}
        # g0 weights: [KT, 128, 512] = whh0 rows
        m["wg0"] = np.ascontiguousarray(
            w_hh0[rows0].T.reshape(KT, 128, 512)).astype(BF16)
        # g1 weights: [2*KT, 128, 512]: k<KT from wih1 (h0 rows), k>=KT whh1
        wg1 = np.concatenate([
            w_ih1[rows1].T.reshape(KT, 128, 512),
            w_hh1[rows1].T.reshape(KT, 128, 512)], axis=0)
        m["wg1"] = np.ascontiguousarray(wg1).astype(BF16)
        # pre-pass x weights: A1 cols in g0-psum order [128, 512]
        m["a1"] = np.ascontiguousarray(A1[:, rows0]).astype(BF16)
        m["xt"] = xt
        # bias for the pre-pass: genc+b0 tiled x4 over the m-tile rows
        # [128, 512]: row p covers sample s = 128m + p -> batch b = p % 32
        m["gencb"] = np.ascontiguousarray(
            np.tile(genc[:, rows0], (4, 1))).astype(BF16)
        # L1 bias (per batch-row): [32, 512]
        m["b1c"] = np.ascontiguousarray(
            np.broadcast_to(b1[rows1], (B, 512))).astype(BF16)
        m["ench"] = np.ascontiguousarray(ench).astype(BF16)  # (128, KT, 32)
        # c init [32, 256] = [enc_r | enc_r]
        ce = enc[:, u:u + 128]
        m["cinit"] = np.ascontiguousarray(
            np.concatenate([ce, ce], axis=1))                # (32, 256) f32
        lw = lin_wT[:, r * VS:(r + 1) * VS]                  # (H, 4000)
        m["linw8"] = np.ascontiguousarray(
            (lw.reshape(KT, 128, VS) * SW)).astype(FP8)
        if use_linb:
            m["linb"] = np.ascontiguousarray(
                lin_b[r * VS:(r + 1) * VS] / DESCALE
            ).astype(np.float32).astype(BF16).reshape(1, VS)
        in_maps.append(m)
    return in_maps, use_linb


def _build(reps=1, use_linb=False, debug_out=False):
    import concourse.tile as tile
    from concourse import bacc, mybir
    from contextlib import ExitStack

    f32 = mybir.dt.float32
    bf16 = mybir.dt.bfloat16
    fp8 = mybir.dt.float8e4
    AF = mybir.ActivationFunctionType
    ALU = mybir.AluOpType
    DR = mybir.MatmulPerfMode.DoubleRow

    nc = bacc.Bacc("TRN2", target_bir_lowering=False, debug=False,
                   num_devices=NCORES)

    d_wg0 = nc.dram_tensor("wg0", [KT, 128, 512], bf16, kind="ExternalInput")
    d_wg1 = nc.dram_tensor("wg1", [2 * KT, 128, 512], bf16, kind="ExternalInput")
    d_a1 = nc.dram_tensor("a1", [128, 512], bf16, kind="ExternalInput")
    d_xt = nc.dram_tensor("xt", [128, S], bf16, kind="ExternalInput")
    d_gencb = nc.dram_tensor("gencb", [128, 512], bf16, kind="ExternalInput")
    d_b1c = nc.dram_tensor("b1c", [B, 512], bf16, kind="ExternalInput")
    d_ench = nc.dram_tensor("ench", [128, KT * B], bf16, kind="ExternalInput")
    d_cinit = nc.dram_tensor("cinit", [B, 256], f32, kind="ExternalInput")
    d_linw8 = nc.dram_tensor("linw8", [KT, 128, VS], fp8, kind="ExternalInput")
    if use_linb:
        d_linb = nc.dram_tensor("linb", [1, VS], bf16, kind="ExternalInput")
    d_out = nc.dram_tensor("out", [S, VS], f32, kind="ExternalOutput")
    if debug_out:
        d_dbg_gxb = nc.dram_tensor("dbg_gxb", [S, 512], bf16, kind="ExternalOutput")
        d_dbg_h1s = nc.dram_tensor("dbg_h1s", [128, KT * S], bf16, kind="ExternalOutput")

    rg = [list(range(NCORES))]

    with tile.TileContext(nc) as tc, ExitStack() as ctx:
        wp = ctx.enter_context(tc.tile_pool(name="w", bufs=1))
        dp = ctx.enter_context(tc.tile_pool(name="db", bufs=6, space="DRAM"))
        hp = ctx.enter_context(tc.tile_pool(name="hx", bufs=3))
        cp = ctx.enter_context(tc.tile_pool(name="ct", bufs=2))
        tp = ctx.enter_context(tc.tile_pool(name="tmp", bufs=4))

        wg0 = wp.tile([128, KT * 512], bf16, name="wg0s")
        wg1 = wp.tile([128, 2 * KT * 512], bf16, name="wg1s")
        a1 = wp.tile([128, 512], bf16, name="a1s")
        xts = wp.tile([128, S], bf16, name="xts")
        gencb = wp.tile([128, 512], bf16, name="gencbs")
        b1c = wp.tile([B, 512], bf16, name="b1cs")
        ench = wp.tile([128, KT * B], bf16, name="enchs")
        h1store = wp.tile([128, KT * S], bf16, name="h1store")
        h8 = wp.tile([128, KT * S], fp8, name="h8store")
        linw8 = wp.tile([128, KT * VS], fp8, name="linw8s")
        if use_linb:
            linb_sb = wp.tile([1, VS], bf16, name="linbs")
            ones = wp.tile([1, 128], bf16, name="ones")

        # small inputs first so tick-0 matmuls aren't queued behind the
        # vocab weights.
        nc.sync.dma_start(a1[:], d_a1[:])
        nc.sync.dma_start(gencb[:], d_gencb[:])
        nc.sync.dma_start(b1c[:], d_b1c[:])
        nc.sync.dma_start(ench[:], d_ench[:])
        nc.sync.dma_start(xts[:], d_xt[:])
        nc.sync.dma_start(
            wg0[:].rearrange("p (k g) -> p k g", k=KT),
            d_wg0[:].rearrange("k p g -> p k g"))
        nc.sync.dma_start(
            wg1[:].rearrange("p (k g) -> p k g", k=2 * KT),
            d_wg1[:].rearrange("k p g -> p k g"))
        nc.sync.dma_start(
            linw8[:].rearrange("p (k v) -> p k v", k=KT),
            d_linw8[:].rearrange("k p v -> p k v"))
        if use_linb:
            nc.sync.dma_start(linb_sb[:], d_linb[:])
            nc.gpsimd.memset(ones[:], 1.0)

        wg0_k = wg0[:].rearrange("p (k g) -> p k g", k=KT)
        wg1_k = wg1[:].rearrange("p (k g) -> p k g", k=2 * KT)
        ench_k = ench[:].rearrange("p (k b) -> p k b", k=KT)
        h1s_k = h1store[:].rearrange("p (k s) -> p k s", k=KT)
        h8_k = h8[:].rearrange("p (k s) -> p k s", k=KT)
        lw_k = linw8[:].rearrange("p (k v) -> p k v", k=KT)

        for _rep in range(reps):
            ct = cp.tile([B, 256], f32, tag="ct")
            nc.sync.dma_start(ct[:], d_cinit[:])
            gxbd = dp.tile([S, 512], bf16, tag="gxbd")

            with tc.tile_pool(name="pg", bufs=2, space="PSUM") as pg, \
                 tc.tile_pool(name="pp", bufs=1, space="PSUM") as ppre, \
                 tc.tile_pool(name="vp", bufs=3, space="PSUM") as vp, \
                 tc.tile_pool(name="lg", bufs=5) as lgp, \
                 tc.tile_pool(name="ob", bufs=2) as obp, \
                 tc.tile_pool(name="ex", bufs=1) as exp_p, \
                 tc.tile_pool(name="gt", bufs=2) as gtp, \
                 tc.tile_pool(name="tot", bufs=1) as totp:

                totals = totp.tile([128, 16], f32, tag="totals")
                lse = totp.tile([128, 16], f32, tag="lse")
                neglse = totp.tile([128, 16], f32, tag="neglse")
                out_tb = d_out[:].rearrange("(b t) v -> t b v", b=B)
                lgt = {}

                def emit_prepass_m(m):
                    """Pre-pass m-tile m: gxb[s, 512] for s in [128m,...)."""
                    M = min(128, S - 128 * m)
                    q = M // 32
                    ps = ppre.tile([128, 512], f32, tag="pp")
                    nc.tensor.matmul(ps[:M], xts[:, m * 128:m * 128 + M],
                                     a1[:], start=True, stop=True)
                    gsb = tp.tile([128, 512], bf16, tag="gpre")
                    nc.vector.tensor_add(gsb[:M], ps[:M], gencb[:M])
                    # rows land sample-major: s = 128m + p
                    nc.sync.dma_start(gxbd[128 * m:128 * m + M, :], gsb[:M])

                def emit_vocab_m(m):
                    """fp8 DoubleRow vocab matmuls + exp for m-tile m."""
                    M = 128 if m < NMT - 1 else S - 128 * (NMT - 1)
                    msl = slice(m * 128, m * 128 + M)
                    lg = lgp.tile([128, VS], bf16, tag="lg")
                    lgt[m] = (lg, M)
                    for c in range(NCHK):
                        ps = vp.tile([128, CHUNK], f32, tag="ps")
                        for j in range(KT // 2):
                            nc.tensor.matmul(
                                ps[:M],
                                h8_k[:, 2 * j:2 * j + 2, msl],
                                lw_k[:, 2 * j:2 * j + 2,
                                     c * CHUNK:(c + 1) * CHUNK],
                                start=(j == 0),
                                stop=(not use_linb and j == KT // 2 - 1),
                                perf_mode=DR)
                        if use_linb:
                            nc.tensor.matmul(
                                ps[:M], ones[0:1, 0:M],
                                linb_sb[0:1, c * CHUNK:(c + 1) * CHUNK],
                                start=False, stop=True)
                        nc.vector.tensor_scalar_mul(
                            lg[:M, c * CHUNK:(c + 1) * CHUNK], ps[:M],
                            DESCALE)
                    ex = exp_p.tile([128, VS], bf16, tag="ex")
                    nc.scalar.activation(ex[:M], lg[:M, :], AF.Exp,
                                         accum_out=totals[:M, m:m + 1])

                def emit_ar_reduce(c0, c1):
                    """AllReduce exp-sums for m in [c0,c1) -> lse/neglse."""
                    nm = c1 - c0
                    ari = dp.tile([128, nm], f32, tag="ari")
                    aro = dp.tile([128, nm], f32, tag="aro",
                                  addr_space="Shared")
                    nc.sync.dma_start(ari[:], totals[:, c0:c1])
                    nc.gpsimd.collective_compute(
                        "AllReduce", ALU.add, replica_groups=rg,
                        ins=[ari[:].opt()], outs=[aro[:].opt()])
                    nc.sync.dma_start(lse[:, c0:c1], aro[:])
                    nc.scalar.activation(lse[:, c0:c1], lse[:, c0:c1], AF.Ln)
                    nc.vector.tensor_scalar_mul(neglse[:, c0:c1],
                                                lse[:, c0:c1], -1.0)

                def emit_ar_out(c0, c1):
                    """log-softmax outputs for m in [c0,c1)."""
                    for m in range(c0, c1):
                        lg, M = lgt.pop(m)
                        q = M // 32
                        for h in range(2):
                            ob = obp.tile([128, VS // 2], f32, tag="ob")
                            if (m + h) % 2 == 0:
                                nc.vector.tensor_scalar(
                                    ob[:M],
                                    lg[:M, h * (VS // 2):(h + 1) * (VS // 2)],
                                    lse[:M, m:m + 1], None, op0=ALU.subtract)
                            else:
                                nc.scalar.activation(
                                    ob[:M],
                                    lg[:M, h * (VS // 2):(h + 1) * (VS // 2)],
                                    AF.Identity, bias=neglse[:M, m:m + 1])
                            dst = out_tb[m * 4:m * 4 + q, :,
                                         h * (VS // 2):(h + 1) * (VS // 2)]
                            nc.sync.dma_start(dst, ob[:M])

                # hx(tau) = [h0(tau-1) | h1(tau-3)]: stationary state,
                # rebuilt from AG(tau-1).  g1 (L1, lag 2) reads its h0 input
                # from hx(tau-1) (2-tick-old AG) so only the whh1/g0 matmuls
                # wait on the freshest AG -- the AG latency hides under the
                # vocab chunks + wih1 matmuls emitted ahead of them.
                hx = None
                hx_prev = None

                emit_prepass_m(0)

                # vocab spread schedule: chunk-pairs of m-tile m at ticks
                # 4m+6+j (j=0..3), two chunks per tick; leftovers go to the
                # tail.  exp-accum emitted with the last pair; ARs at fixed
                # ticks after their m-tiles complete.
                cast_queue = []
                vocab_sched = {}
                for m in range(NMT):
                    for j in range(4):
                        tick = 4 * m + 7 + j
                        if tick < T + 2:
                            vocab_sched.setdefault(tick, []).append((m, j))

                def emit_vocab_pair(m, j):
                    """Chunks 2j, 2j+1 of m-tile m; exp-accum after the last."""
                    M = 128 if m < NMT - 1 else S - 128 * (NMT - 1)
                    msl = slice(m * 128, m * 128 + M)
                    if j == 0:
                        lg_new = lgp.tile([128, VS], bf16, tag="lg")
                        lgt[m] = (lg_new, M)
                    lg, _ = lgt[m]
                    for c in (2 * j, 2 * j + 1):
                        ps = vp.tile([128, CHUNK], f32, tag="ps")
                        for jj in range(KT // 2):
                            nc.tensor.matmul(
                                ps[:M],
                                h8_k[:, 2 * jj:2 * jj + 2, msl],
                                lw_k[:, 2 * jj:2 * jj + 2,
                                     c * CHUNK:(c + 1) * CHUNK],
                                start=(jj == 0),
                                stop=(not use_linb and jj == KT // 2 - 1),
                                perf_mode=DR)
                        if use_linb:
                            nc.tensor.matmul(
                                ps[:M], ones[0:1, 0:M],
                                linb_sb[0:1, c * CHUNK:(c + 1) * CHUNK],
                                start=False, stop=True)
                        nc.vector.tensor_scalar_mul(
                            lg[:M, c * CHUNK:(c + 1) * CHUNK], ps[:M],
                            DESCALE)
                    if j == 3:
                        ex = exp_p.tile([128, VS], bf16, tag="ex")
                        nc.scalar.activation(ex[:M], lg[:M, :], AF.Exp,
                                             accum_out=totals[:M, m:m + 1])

                for tau in range(T + 2):
                    has_g0 = tau < T            # L0 step tau
                    has_g1 = tau >= 2           # L1 step tau-2

                    # ---- AG-independent PE work first: vocab chunks ----
                    if tau == 25:
                        emit_ar_out(0, 4)
                    elif tau == 41:
                        emit_ar_out(4, 8)
                    for (m, j) in vocab_sched.get(tau, ()):
                        emit_vocab_pair(m, j)

                    if not (first_hx := tau == 0):
                        hx_k = hx[:].rearrange("p (k h b) -> p k h b",
                                               k=KT, h=2)
                    if hx_prev is not None:
                        hxp_k = hx_prev[:].rearrange("p (k h b) -> p k h b",
                                                     k=KT, h=2)

                    # ---- g1: wih1 part (2-tick-old h0) then whh1 part ----
                    if has_g1:
                        g1 = pg.tile([B, 512], f32, tag="g1")
                        for k in range(KT):
                            nc.tensor.matmul(g1[:], hxp_k[:, k, 0, :],
                                             wg1_k[:, k, :],
                                             start=(k == 0), stop=False)
                        for k in range(KT):
                            nc.tensor.matmul(g1[:], hx_k[:, k, 1, :],
                                             wg1_k[:, KT + k, :],
                                             start=False, stop=(k == KT - 1))

                    # ---- g0 (1-tick-old h0) ----
                    if has_g0:
                        g0 = pg.tile([B, 512], f32, tag="g0")
                        if tau == 0:
                            for k in range(KT):
                                nc.tensor.matmul(g0[:], ench_k[:, k, :],
                                                 wg0_k[:, k, :],
                                                 start=(k == 0),
                                                 stop=(k == KT - 1))
                        else:
                            for k in range(KT):
                                nc.tensor.matmul(g0[:], hx_k[:, k, 0, :],
                                                 wg0_k[:, k, :],
                                                 start=(k == 0),
                                                 stop=(k == KT - 1))

                    # ---- psum -> SBUF gates with bias add ----
                    gates = gtp.tile([B, 1024], f32, tag="gates")
                    gv = gates[:].rearrange("b (q l c) -> b q l c", q=4, l=2)
                    if has_g0:
                        gxt = tp.tile([B, 512], bf16, tag="gxt")
                        nc.sync.dma_start(gxt[:], gxbd[32 * tau:32 * tau + 32, :])
                        nc.vector.tensor_add(
                            gv[:, :, 0, :],
                            g0[:].rearrange("b (q c) -> b q c", q=4),
                            gxt[:].rearrange("b (q c) -> b q c", q=4))
                    if has_g1:
                        nc.vector.tensor_add(
                            gv[:, :, 1, :],
                            g1[:].rearrange("b (q c) -> b q c", q=4),
                            b1c[:].rearrange("b (q c) -> b q c", q=4))

                    # ---- fused cells: h-pair [h0(tau) | h1(tau-2)] ----
                    if has_g0 and not has_g1:
                        ssl = gv[:, 0:3, 0, :]
                        tsl = gv[:, 3, 0, :]
                        csl = ct[:, 0:128]
                        hw = 128
                    elif has_g1 and not has_g0:
                        ssl = gv[:, 0:3, 1, :]
                        tsl = gv[:, 3, 1, :]
                        csl = ct[:, 128:256]
                        hw = 128
                    else:
                        ssl = gates[:, 0:768]
                        tsl = gates[:, 768:1024]
                        csl = ct[:]
                        hw = 256
                    sg = tp.tile([B, 3 * hw], bf16, tag="sg")
                    nc.scalar.activation(sg[:], ssl, AF.Sigmoid)
                    tg = tp.tile([B, hw], bf16, tag="tg")
                    nc.scalar.activation(tg[:], tsl, AF.Tanh)
                    t1 = tp.tile([B, hw], f32, tag="t1")
                    nc.vector.tensor_mul(t1[:], sg[:, 0:hw], tg[:])
                    ctn = cp.tile([B, 256], f32, tag="ct")
                    if hw == 256:
                        nc.vector.tensor_mul(ctn[:], sg[:, hw:2 * hw], csl)
                        nc.vector.tensor_add(ctn[:], ctn[:], t1[:])
                        cview = ctn[:]
                    else:
                        l0only = has_g0
                        half = slice(0, 128) if l0only else slice(128, 256)
                        other = slice(128, 256) if l0only else slice(0, 128)
                        nc.vector.tensor_mul(ctn[:, half], sg[:, hw:2 * hw],
                                             csl)
                        nc.vector.tensor_add(ctn[:, half], ctn[:, half],
                                             t1[:])
                        nc.vector.tensor_copy(ctn[:, other], ct[:, other])
                        cview = ctn[:, half]
                    ct = ctn
                    th = tp.tile([B, hw], bf16, tag="th")
                    nc.scalar.activation(th[:], cview, AF.Tanh)
                    hb = tp.tile([B, hw], bf16, tag="hb")
                    nc.vector.tensor_mul(hb[:], sg[:, 2 * hw:3 * hw], th[:])

                    # ---- transpose + AllGather [h0(tau) | h1(tau-2)] ----
                    tb = tp.tile([B, hw], bf16, tag="tb")
                    nc.vector.transpose(tb[:], hb[:])
                    nhalf = hw // 128
                    bi = dp.tile([128, 32 * nhalf], bf16, tag="bi")
                    nc.sync.dma_start(
                        bi[:].rearrange("(q i) (l b) -> i l q b",
                                        q=4, l=nhalf),
                        tb[:].rearrange("i (l q b) -> i l q b",
                                        l=nhalf, q=4))
                    bo = dp.tile([NCORES, 128, 32 * nhalf], bf16, tag="bo",
                                 addr_space="Shared")
                    nc.gpsimd.collective_compute(
                        "AllGather", ALU.bypass, replica_groups=rg,
                        ins=[bi[:].opt()], outs=[bo[:].opt()])
                    bo_v = bo[:].rearrange("k p (l b) -> p k l b", l=nhalf)

                    if tau < T + 1:
                        # hx(tau+1) = [h0(tau) | h1(tau-2)]
                        hxn = hp.tile([128, KT * 2 * 32], bf16, tag="hx")
                        hxn_k = hxn[:].rearrange("p (k h b) -> p k h b",
                                                 k=KT, h=2)
                        if has_g0 and has_g1:
                            nc.sync.dma_start(
                                hxn[:].rearrange("p (k h b) -> p k h b",
                                                 k=KT, h=2),
                                bo_v[:])
                        elif has_g0:
                            nc.sync.dma_start(hxn_k[:, :, 0, :],
                                              bo_v[:, :, 0, :])
                            if tau == 1:
                                # g1(2) needs h1(-1) = enc
                                nc.sync.dma_start(hxn_k[:, :, 1, :],
                                                  ench_k[:])
                        else:
                            nc.sync.dma_start(hxn_k[:, :, 1, :],
                                              bo_v[:, :, 0, :])
                            # keep h0 half from previous hx (not used again)
                        hx_prev = hx
                        hx = hxn
                    else:
                        hx_prev = hx

                    # ---- h1 store; fp8 cast deferred 2 ticks so the DVE
                    # queue is not head-blocked waiting on the fresh AG ----
                    if has_g1:
                        t1_ = tau - 2
                        sl = slice(t1_ * 32, (t1_ + 1) * 32)
                        li = 1 if has_g0 else 0
                        nc.sync.dma_start(h1s_k[:, :, sl], bo_v[:, :, li, :])
                        cast_queue.append(sl)
                    if len(cast_queue) > 1:
                        slq = cast_queue.pop(0)
                        nc.vector.tensor_scalar_mul(
                            h8_k[:, :, slq], h1s_k[:, :, slq], SH)

                    # ---- AR reduces: after the AG so the gpsimd FIFO runs
                    # the AG first ----
                    if tau == 23:
                        emit_ar_reduce(0, 4)
                    elif tau == 39:
                        emit_ar_reduce(4, 8)

                    # ---- pre-pass interleave ----
                    if tau < NMT - 1:
                        emit_prepass_m(tau + 1)

                # tail: leftover casts + vocab pairs + ARs + outputs
                for slq in cast_queue:
                    nc.vector.tensor_scalar_mul(
                        h8_k[:, :, slq], h1s_k[:, :, slq], SH)
                cast_queue = []
                for m in range(NMT):
                    for j in range(4):
                        tick = 4 * m + 7 + j
                        if tick >= T + 2:
                            emit_vocab_pair(m, j)
                emit_ar_reduce(8, 13)
                emit_ar_out(8, 13)
                if debug_out:
                    nc.sync.dma_start(d_dbg_gxb[:], gxbd[:])
                    nc.sync.dma_start(d_dbg_h1s[:], h1store[:])

    nc.compile()
    return nc


def _get_nc(reps=1, use_linb=False, debug_out=False):
    key = ("nc", reps, use_linb, debug_out)
    if key not in _BUILD_CACHE:
        _BUILD_CACHE[key] = _build(reps, use_linb, debug_out)
    return _BUILD_CACHE[key]


def run(inputs, trace=False, reps=1, debug_out=False):
    from concourse.bass_utils import run_bass_kernel_spmd

    in_maps, use_linb = _host_prep(inputs)
    nc = _get_nc(reps, use_linb, debug_out)
    res = run_bass_kernel_spmd(nc, in_maps, core_ids=list(range(NCORES)),
                               trace=trace)
    full = np.empty((S, V), np.float32)
    for r in range(NCORES):
        full[:, r * VS:(r + 1) * VS] = res.results[r]["out"]
    return full, res


def kernel(**inputs):
    full, _ = run(inputs)
    return full


# revision 8
# speedup vs baseline: 1.0544x; 1.0544x over previous
"""Trainium2 Bass kernel for nn_Decoder_25718264168590 (v4).

2-layer LSTM decoder (B=32, T=50, H=1024, E=128) + vocab projection
(V=32000) + log_softmax, on 8 NeuronCores.

This environment executes instructions at ~60-95 us each (flat in data
size), so the design minimizes instruction count on the PE chain and
collective/barrier count:

- Gate-sharded recurrence: core r owns h-units [128r, 128r+128) of both
  layers.  Layer 1 lags layer 0 by one tick; both layers' gates are
  computed per tick as two 512-col PSUM groups: g0 (8 k-tile matmuls
  over h0) and g1 (16 k-tiles over [h0; h1]).  24 matmuls/tick.
- The x-path (target @ A1) + enc-path + biases are folded into a
  26-matmul PRE-PASS producing a SBUF-resident gxb table [32, T*512];
  per tick the bias/x contribution is added by one DVE op reading PSUM.
- ONE AllGather per tick ships [h0(tau)^T | h1(tau-2)^T] = [128, 64]
  bf16 (edge ticks ship [128, 32]).  52 barriers total vs 104, and the
  L1 lag of TWO ticks means the wih1 matmuls depend only on 2-tick-old
  AG data: per tick the PE chain runs [vocab chunks | wih1] (no wait)
  before [whh1 | g0] (1-tick-old AG), hiding most of the AG latency.
- The lse AllReduce is split: the reduce fires 2 ticks before the
  output pass consumes lse, so the AR latency never head-blocks the
  DVE queue.
- Both cells' nonlinearities are fused: per-core gate columns are
  arranged [i0 i1 f0 f1 o0 o1 g0 g1] (128 each), so one sigmoid over
  [32, 768]-strided, one tanh over [32, 256], one tanh over c [32,256],
  and 4 wide DVE ops update both layers at once.
- Vocab projection: vocab-sharded (4000 cols/core), fp8e4 DoubleRow,
  spread as 2-chunk pairs into EVERY tick (scheduled >=2 ticks after
  their h1 slices land so they never wait on the newest AG); chunked
  AllReduce for the log-sum-exp; outputs written as log-softmax.
"""

import sys

for _p in ("/opt/trn_rl_repo",):
    if _p not in sys.path:
        sys.path.insert(0, _p)

import numpy as np
import ml_dtypes

B, T, H, E, V = 32, 50, 1024, 128, 32000
NCORES = 8
VS = V // NCORES          # 4000 vocab cols per core
S = B * T                 # 1600 samples, t-major on device: s = t*32 + b
KT = H // 128             # 8 k-tiles of hidden per layer
NMT = 13                  # sample m-tiles in vocab phase (12*128 + 64)
NCHK = 8                  # vocab col chunks per core (8 * 500)
CHUNK = VS // NCHK        # 500
AR_CHUNKS = ((0, 4), (4, 8), (8, 12), (12, 13))

SW = 2048.0               # fp8 weight scale (|w|<=0.1 -> <=204.8)
SH = 128.0                # fp8 h1 scale (|h|<=1 -> <=128)
DESCALE = 1.0 / (SW * SH)

BF16 = ml_dtypes.bfloat16
FP8 = ml_dtypes.float8_e4m3

_BUILD_CACHE = {}

# gate-column arrangement inside each core's 1024 cols:
# [i0 i1 f0 f1 o0 o1 g0 g1], 128 each.  PSUM g0 holds L0's (i0 f0 o0 g0)
# contiguously; PSUM g1 holds L1's (i1 f1 o1 g1).  The DVE add that
# moves PSUM->SBUF scatters into the interleaved arrangement.
# torch gate order in weights: i, f, g, o.
G_L0 = (0, 1, 3, 2)       # torch (i,f,g,o) -> psum order (i, f, o, g)
G_L1 = (0, 1, 3, 2)


def _host_prep(inputs):
    enc = np.asarray(inputs["enc_output"], np.float32)       # (B, H)
    target = np.asarray(inputs["target"], np.float32)        # (B, T, E)
    proj_w = np.asarray(inputs["proj_w"], np.float32)        # (E, H+E)
    proj_b = np.asarray(inputs["proj_b"], np.float32)        # (E,)
    w_ih0 = np.asarray(inputs["w_ih0"], np.float32)          # (4H, E)
    w_hh0 = np.asarray(inputs["w_hh0"], np.float32)          # (4H, H)
    b0 = np.asarray(inputs["b_ih0"], np.float32) + np.asarray(inputs["b_hh0"], np.float32)
    w_ih1 = np.asarray(inputs["w_ih1"], np.float32)          # (4H, H)
    w_hh1 = np.asarray(inputs["w_hh1"], np.float32)          # (4H, H)
    b1 = np.asarray(inputs["b_ih1"], np.float32) + np.asarray(inputs["b_hh1"], np.float32)
    lin_w = np.asarray(inputs["lin_w"], np.float32)          # (V, H)
    lin_b = np.asarray(inputs["lin_b"], np.float32)          # (V,)

    P1 = proj_w[:, :E].T                                     # (E, E)
    P2 = proj_w[:, E:].T                                     # (H, E)
    A1 = P1 @ w_ih0.T                                        # (E, 4H) x-path fold
    genc = (enc @ P2 + proj_b) @ w_ih0.T + b0                # (B, 4H) enc fold + b0

    # t-major input features: xt[e, t*32+b] = target[b, t, e]
    xt = np.ascontiguousarray(
        target.transpose(1, 0, 2).reshape(S, E).T).astype(BF16)   # (128, 1600)

    # exchange-layout encoder init: ench[p, k, b] = enc[b, k*128+p]
    ench = np.ascontiguousarray(
        enc.T.reshape(KT, 128, B).transpose(1, 0, 2))             # (128, KT, B)

    lin_wT = lin_w.T                                         # (H, V)
    use_linb = bool(np.any(lin_b != 0.0))

    in_maps = []
    for r in range(NCORES):
        u = r * 128                                          # h-unit base
        rows0 = np.concatenate(
            [np.arange(128) + g * H + u for g in G_L0])      # L0's 512 gate rows
        rows1 = np.concatenate(
            [np.arange(128) + g * H + u for g in G_L1])      # L1's 512 gate rows
        m = {}
        # g-gate columns are pre-scaled x2 so tanh(g) = 2*sigmoid(2g)-1
        # falls out of the single wide sigmoid.
        def g2(a):
            a = a.copy()
            a[..., 384:512] *= 2.0
            return a
        # g0 weights: [KT, 128, 512] = whh0 rows
        m["wg0"] = np.ascontiguousarray(
            g2(w_hh0[rows0].T.reshape(KT, 128, 512))).astype(BF16)
        # g1 weights: [2*KT, 128, 512]: k<KT from wih1 (h0 rows), k>=KT whh1
        wg1 = np.concatenate([
            w_ih1[rows1].T.reshape(KT, 128, 512),
            w_hh1[rows1].T.reshape(KT, 128, 512)], axis=0)
        m["wg1"] = np.ascontiguousarray(g2(wg1)).astype(BF16)
        # pre-pass x weights: A1 cols in g0-psum order [128, 512]
        m["a1"] = np.ascontiguousarray(g2(A1[:, rows0])).astype(BF16)
        m["xt"] = xt
        # bias for the pre-pass: genc+b0 tiled x4 over the m-tile rows
        # [128, 512]: row p covers sample s = 128m + p -> batch b = p % 32
        m["gencb"] = np.ascontiguousarray(
            g2(np.tile(genc[:, rows0], (4, 1)))).astype(BF16)
        # L1 bias (per batch-row): [32, 512]
        m["b1c"] = np.ascontiguousarray(
            g2(np.broadcast_to(b1[rows1], (B, 512)).copy())).astype(BF16)
        m["ench"] = np.ascontiguousarray(ench).astype(BF16)  # (128, KT, 32)
        # c init [32, 256] = [enc_r | enc_r]
        ce = enc[:, u:u + 128]
        m["cinit"] = np.ascontiguousarray(
            np.concatenate([ce, ce], axis=1))                # (32, 256) f32
        lw = lin_wT[:, r * VS:(r + 1) * VS]                  # (H, 4000)
        m["linw8"] = np.ascontiguousarray(
            (lw.reshape(KT, 128, VS) * SW)).astype(FP8)
        if use_linb:
            m["linb"] = np.ascontiguousarray(
                lin_b[r * VS:(r + 1) * VS] / DESCALE
            ).astype(np.float32).astype(BF16).reshape(1, VS)
        in_maps.append(m)
    return in_maps, use_linb


def _build(reps=1, use_linb=False, debug_out=False):
    import concourse.tile as tile
    from concourse import bacc, mybir
    from contextlib import ExitStack

    f32 = mybir.dt.float32
    bf16 = mybir.dt.bfloat16
    fp8 = mybir.dt.float8e4
    AF = mybir.ActivationFunctionType
    ALU = mybir.AluOpType
    DR = mybir.MatmulPerfMode.DoubleRow

    nc = bacc.Bacc("TRN2", target_bir_lowering=False, debug=False,
                   num_devices=NCORES)

    d_wg0 = nc.dram_tensor("wg0", [KT, 128, 512], bf16, kind="ExternalInput")
    d_wg1 = nc.dram_tensor("wg1", [2 * KT, 128, 512], bf16, kind="ExternalInput")
    d_a1 = nc.dram_tensor("a1", [128, 512], bf16, kind="ExternalInput")
    d_xt = nc.dram_tensor("xt", [128, S], bf16, kind="ExternalInput")
    d_gencb = nc.dram_tensor("gencb", [128, 512], bf16, kind="ExternalInput")
    d_b1c = nc.dram_tensor("b1c", [B, 512], bf16, kind="ExternalInput")
    d_ench = nc.dram_tensor("ench", [128, KT * B], bf16, kind="ExternalInput")
    d_cinit = nc.dram_tensor("cinit", [B, 256], f32, kind="ExternalInput")
    d_linw8 = nc.dram_tensor("linw8", [KT, 128, VS], fp8, kind="ExternalInput")
    if use_linb:
        d_linb = nc.dram_tensor("linb", [1, VS], bf16, kind="ExternalInput")
    d_out = nc.dram_tensor("out", [S, VS], f32, kind="ExternalOutput")
    if debug_out:
        d_dbg_gxb = nc.dram_tensor("dbg_gxb", [S, 512], bf16, kind="ExternalOutput")
        d_dbg_h1s = nc.dram_tensor("dbg_h1s", [128, KT * S], bf16, kind="ExternalOutput")

    rg = [list(range(NCORES))]

    with tile.TileContext(nc) as tc, ExitStack() as ctx:
        wp = ctx.enter_context(tc.tile_pool(name="w", bufs=1))
        dp = ctx.enter_context(tc.tile_pool(name="db", bufs=6, space="DRAM"))
        hp = ctx.enter_context(tc.tile_pool(name="hx", bufs=3))
        cp = ctx.enter_context(tc.tile_pool(name="ct", bufs=2))
        tp = ctx.enter_context(tc.tile_pool(name="tmp", bufs=2))

        wg0 = wp.tile([128, KT * 512], bf16, name="wg0s")
        wg1 = wp.tile([128, 2 * KT * 512], bf16, name="wg1s")
        a1 = wp.tile([128, 512], bf16, name="a1s")
        xts = wp.tile([128, S], bf16, name="xts")
        gencb = wp.tile([128, 512], bf16, name="gencbs")
        b1c = wp.tile([B, 512], bf16, name="b1cs")
        ench = wp.tile([128, KT * B], bf16, name="enchs")
        h1store = wp.tile([128, KT * S], bf16, name="h1store")
        h8 = wp.tile([128, KT * S], fp8, name="h8store")
        linw8 = wp.tile([128, KT * VS], fp8, name="linw8s")
        if use_linb:
            linb_sb = wp.tile([1, VS], bf16, name="linbs")
            ones = wp.tile([1, 128], bf16, name="ones")

        # small inputs first so tick-0 matmuls aren't queued behind the
        # vocab weights.
        nc.sync.dma_start(a1[:], d_a1[:])
        nc.sync.dma_start(gencb[:], d_gencb[:])
        nc.sync.dma_start(b1c[:], d_b1c[:])
        nc.sync.dma_start(ench[:], d_ench[:])
        nc.sync.dma_start(xts[:], d_xt[:])
        nc.sync.dma_start(
            wg0[:].rearrange("p (k g) -> p k g", k=KT),
            d_wg0[:].rearrange("k p g -> p k g"))
        nc.sync.dma_start(
            wg1[:].rearrange("p (k g) -> p k g", k=2 * KT),
            d_wg1[:].rearrange("k p g -> p k g"))
        nc.sync.dma_start(
            linw8[:].rearrange("p (k v) -> p k v", k=KT),
            d_linw8[:].rearrange("k p v -> p k v"))
        if use_linb:
            nc.sync.dma_start(linb_sb[:], d_linb[:])
            nc.gpsimd.memset(ones[:], 1.0)

        wg0_k = wg0[:].rearrange("p (k g) -> p k g", k=KT)
        wg1_k = wg1[:].rearrange("p (k g) -> p k g", k=2 * KT)
        ench_k = ench[:].rearrange("p (k b) -> p k b", k=KT)
        h1s_k = h1store[:].rearrange("p (k s) -> p k s", k=KT)
        h8_k = h8[:].rearrange("p (k s) -> p k s", k=KT)
        lw_k = linw8[:].rearrange("p (k v) -> p k v", k=KT)

        for _rep in range(reps):
            ct = cp.tile([B, 256], f32, tag="ct")
            nc.sync.dma_start(ct[:], d_cinit[:])
            gxbd = dp.tile([S, 512], bf16, tag="gxbd")

            with tc.tile_pool(name="pg", bufs=2, space="PSUM") as pg, \
                 tc.tile_pool(name="pp", bufs=1, space="PSUM") as ppre, \
                 tc.tile_pool(name="vp", bufs=3, space="PSUM") as vp, \
                 tc.tile_pool(name="lg", bufs=5) as lgp, \
                 tc.tile_pool(name="ob", bufs=2) as obp, \
                 tc.tile_pool(name="ex", bufs=1) as exp_p, \
                 tc.tile_pool(name="gt", bufs=2) as gtp, \
                 tc.tile_pool(name="tot", bufs=1) as totp:

                totals = totp.tile([128, 16], f32, tag="totals")
                lse = totp.tile([128, 16], f32, tag="lse")
                neglse = totp.tile([128, 16], f32, tag="neglse")
                out_tb = d_out[:].rearrange("(b t) v -> t b v", b=B)
                lgt = {}

                def emit_prepass_m(m):
                    """Pre-pass m-tile m: gxb[s, 512] for s in [128m,...)."""
                    M = min(128, S - 128 * m)
                    q = M // 32
                    ps = ppre.tile([128, 512], f32, tag="pp")
                    nc.tensor.matmul(ps[:M], xts[:, m * 128:m * 128 + M],
                                     a1[:], start=True, stop=True)
                    gsb = tp.tile([128, 512], bf16, tag="gpre")
                    nc.vector.tensor_add(gsb[:M], ps[:M], gencb[:M])
                    # rows land sample-major: s = 128m + p
                    nc.sync.dma_start(gxbd[128 * m:128 * m + M, :], gsb[:M])

                def emit_vocab_m(m):
                    """fp8 DoubleRow vocab matmuls + exp for m-tile m."""
                    M = 128 if m < NMT - 1 else S - 128 * (NMT - 1)
                    msl = slice(m * 128, m * 128 + M)
                    lg = lgp.tile([128, VS], bf16, tag="lg")
                    lgt[m] = (lg, M)
                    for c in range(NCHK):
                        ps = vp.tile([128, CHUNK], f32, tag="ps")
                        for j in range(KT // 2):
                            nc.tensor.matmul(
                                ps[:M],
                                h8_k[:, 2 * j:2 * j + 2, msl],
                                lw_k[:, 2 * j:2 * j + 2,
                                     c * CHUNK:(c + 1) * CHUNK],
                                start=(j == 0),
                                stop=(not use_linb and j == KT // 2 - 1),
                                perf_mode=DR)
                        if use_linb:
                            nc.tensor.matmul(
                                ps[:M], ones[0:1, 0:M],
                                linb_sb[0:1, c * CHUNK:(c + 1) * CHUNK],
                                start=False, stop=True)
                        nc.vector.tensor_scalar_mul(
                            lg[:M, c * CHUNK:(c + 1) * CHUNK], ps[:M],
                            DESCALE)
                    ex = exp_p.tile([128, VS], bf16, tag="ex")
                    nc.scalar.activation(ex[:M], lg[:M, :], AF.Exp,
                                         accum_out=totals[:M, m:m + 1])

                def emit_ar_reduce(c0, c1):
                    """AllReduce exp-sums for m in [c0,c1) -> lse/neglse."""
                    nm = c1 - c0
                    ari = dp.tile([128, nm], f32, tag="ari")
                    aro = dp.tile([128, nm], f32, tag="aro",
                                  addr_space="Shared")
                    nc.sync.dma_start(ari[:], totals[:, c0:c1])
                    nc.gpsimd.collective_compute(
                        "AllReduce", ALU.add, replica_groups=rg,
                        ins=[ari[:].opt()], outs=[aro[:].opt()])
                    nc.sync.dma_start(lse[:, c0:c1], aro[:])
                    nc.scalar.activation(lse[:, c0:c1], lse[:, c0:c1], AF.Ln)
                    nc.vector.tensor_scalar_mul(neglse[:, c0:c1],
                                                lse[:, c0:c1], -1.0)

                def emit_ar_out(c0, c1):
                    """log-softmax outputs for m in [c0,c1)."""
                    for m in range(c0, c1):
                        lg, M = lgt.pop(m)
                        q = M // 32
                        for h in range(2):
                            ob = obp.tile([128, VS // 2], f32, tag="ob")
                            if (m + h) % 2 == 0:
                                nc.vector.tensor_scalar(
                                    ob[:M],
                                    lg[:M, h * (VS // 2):(h + 1) * (VS // 2)],
                                    lse[:M, m:m + 1], None, op0=ALU.subtract)
                            else:
                                nc.scalar.activation(
                                    ob[:M],
                                    lg[:M, h * (VS // 2):(h + 1) * (VS // 2)],
                                    AF.Identity, bias=neglse[:M, m:m + 1])
                            dst = out_tb[m * 4:m * 4 + q, :,
                                         h * (VS // 2):(h + 1) * (VS // 2)]
                            nc.sync.dma_start(dst, ob[:M])

                # hx(tau) = [h0(tau-1) | h1(tau-3)]: stationary state,
                # rebuilt from AG(tau-1).  g1 (L1, lag 2) reads its h0 input
                # from hx(tau-1) (2-tick-old AG) so only the whh1/g0 matmuls
                # wait on the freshest AG -- the AG latency hides under the
                # vocab chunks + wih1 matmuls emitted ahead of them.
                hx = None
                hx_prev = None

                emit_prepass_m(0)

                # vocab spread schedule: chunk-pairs of m-tile m at ticks
                # 4m+6+j (j=0..3), two chunks per tick; leftovers go to the
                # tail.  exp-accum emitted with the last pair; ARs at fixed
                # ticks after their m-tiles complete.
                cast_queue = []
                vocab_sched = {}
                for m in range(NMT):
                    for j in range(4):
                        tick = 4 * m + 7 + j
                        if tick < T + 2:
                            vocab_sched.setdefault(tick, []).append((m, j))

                def emit_vocab_pair(m, j):
                    """Chunks 2j, 2j+1 of m-tile m; exp-accum after the last."""
                    M = 128 if m < NMT - 1 else S - 128 * (NMT - 1)
                    msl = slice(m * 128, m * 128 + M)
                    if j == 0:
                        lg_new = lgp.tile([128, VS], bf16, tag="lg")
                        lgt[m] = (lg_new, M)
                    lg, _ = lgt[m]
                    for c in (2 * j, 2 * j + 1):
                        ps = vp.tile([128, CHUNK], f32, tag="ps")
                        for jj in range(KT // 2):
                            nc.tensor.matmul(
                                ps[:M],
                                h8_k[:, 2 * jj:2 * jj + 2, msl],
                                lw_k[:, 2 * jj:2 * jj + 2,
                                     c * CHUNK:(c + 1) * CHUNK],
                                start=(jj == 0),
                                stop=(not use_linb and jj == KT // 2 - 1),
                                perf_mode=DR)
                        if use_linb:
                            nc.tensor.matmul(
                                ps[:M], ones[0:1, 0:M],
                                linb_sb[0:1, c * CHUNK:(c + 1) * CHUNK],
                                start=False, stop=True)
                        nc.vector.tensor_scalar_mul(
                            lg[:M, c * CHUNK:(c + 1) * CHUNK], ps[:M],
                            DESCALE)
                    if j == 3:
                        ex = exp_p.tile([128, VS], bf16, tag="ex")
                        nc.scalar.activation(ex[:M], lg[:M, :], AF.Exp,
                                             accum_out=totals[:M, m:m + 1])

                for tau in range(T + 2):
                    has_g0 = tau < T            # L0 step tau
                    has_g1 = tau >= 2           # L1 step tau-2

                    # ---- AG-independent PE work first: vocab chunks ----
                    if tau == 25:
                        emit_ar_out(0, 4)
                    elif tau == 41:
                        emit_ar_out(4, 8)
                    for (m, j) in vocab_sched.get(tau, ()):
                        emit_vocab_pair(m, j)

                    if not (first_hx := tau == 0):
                        hx_k = hx[:].rearrange("p (k h b) -> p k h b",
                                               k=KT, h=2)
                    if hx_prev is not None:
                        hxp_k = hx_prev[:].rearrange("p (k h b) -> p k h b",
                                                     k=KT, h=2)

                    # ---- g1: wih1 part (2-tick-old h0) then whh1 part ----
                    if has_g1:
                        g1 = pg.tile([B, 512], f32, tag="g1")
                        for k in range(KT):
                            nc.tensor.matmul(g1[:], hxp_k[:, k, 0, :],
                                             wg1_k[:, k, :],
                                             start=(k == 0), stop=False)
                        for k in range(KT):
                            nc.tensor.matmul(g1[:], hx_k[:, k, 1, :],
                                             wg1_k[:, KT + k, :],
                                             start=False, stop=(k == KT - 1))

                    # ---- g0 (1-tick-old h0) ----
                    if has_g0:
                        g0 = pg.tile([B, 512], f32, tag="g0")
                        if tau == 0:
                            for k in range(KT):
                                nc.tensor.matmul(g0[:], ench_k[:, k, :],
                                                 wg0_k[:, k, :],
                                                 start=(k == 0),
                                                 stop=(k == KT - 1))
                        else:
                            for k in range(KT):
                                nc.tensor.matmul(g0[:], hx_k[:, k, 0, :],
                                                 wg0_k[:, k, :],
                                                 start=(k == 0),
                                                 stop=(k == KT - 1))

                    # ---- psum -> SBUF gates with bias add ----
                    gates = gtp.tile([B, 1024], f32, tag="gates")
                    gv = gates[:].rearrange("b (q l c) -> b q l c", q=4, l=2)
                    if has_g0:
                        gxt = tp.tile([B, 512], bf16, tag="gxt")
                        nc.sync.dma_start(gxt[:], gxbd[32 * tau:32 * tau + 32, :])
                        nc.vector.tensor_add(
                            gv[:, :, 0, :],
                            g0[:].rearrange("b (q c) -> b q c", q=4),
                            gxt[:].rearrange("b (q c) -> b q c", q=4))
                    if has_g1:
                        nc.vector.tensor_add(
                            gv[:, :, 1, :],
                            g1[:].rearrange("b (q c) -> b q c", q=4),
                            b1c[:].rearrange("b (q c) -> b q c", q=4))

                    # ---- fused cells: h-pair [h0(tau) | h1(tau-2)] ----
                    if has_g0 and not has_g1:
                        ssl = gv[:, :, 0, :]
                        csl = ct[:, 0:128]
                        hw = 128
                    elif has_g1 and not has_g0:
                        ssl = gv[:, :, 1, :]
                        csl = ct[:, 128:256]
                        hw = 128
                    else:
                        ssl = gates[:]
                        csl = ct[:]
                        hw = 256
                    sg = tp.tile([B, 4 * hw], f32, tag="sg")
                    nc.scalar.activation(sg[:], ssl, AF.Sigmoid)
                    tg = tp.tile([B, hw], bf16, tag="tg")
                    nc.vector.tensor_scalar(tg[:], sg[:, 3 * hw:4 * hw],
                                            2.0, -1.0, op0=ALU.mult,
                                            op1=ALU.add)
                    t1 = tp.tile([B, hw], f32, tag="t1")
                    nc.vector.tensor_mul(t1[:], sg[:, 0:hw], tg[:])
                    ctn = cp.tile([B, 256], f32, tag="ct")
                    if hw == 256:
                        nc.vector.tensor_mul(ctn[:], sg[:, hw:2 * hw], csl)
                        nc.vector.tensor_add(ctn[:], ctn[:], t1[:])
                        cview = ctn[:]
                    else:
                        l0only = has_g0
                        half = slice(0, 128) if l0only else slice(128, 256)
                        other = slice(128, 256) if l0only else slice(0, 128)
                        nc.vector.tensor_mul(ctn[:, half], sg[:, hw:2 * hw],
                                             csl)
                        nc.vector.tensor_add(ctn[:, half], ctn[:, half],
                                             t1[:])
                        nc.vector.tensor_copy(ctn[:, other], ct[:, other])
                        cview = ctn[:, half]
                    ct = ctn
                    th = tp.tile([B, hw], bf16, tag="th")
                    nc.scalar.activation(th[:], cview, AF.Tanh)
                    hb = tp.tile([B, hw], bf16, tag="hb")
                    nc.vector.tensor_mul(hb[:], sg[:, 2 * hw:3 * hw], th[:])

                    # ---- transpose + AllGather [h0(tau) | h1(tau-2)] ----
                    tb = tp.tile([B, hw], bf16, tag="tb")
                    nc.vector.transpose(tb[:], hb[:])
                    nhalf = hw // 128
                    bi = dp.tile([128, 32 * nhalf], bf16, tag="bi")
                    nc.sync.dma_start(
                        bi[:].rearrange("(q i) (l b) -> i l q b",
                                        q=4, l=nhalf),
                        tb[:].rearrange("i (l q b) -> i l q b",
                                        l=nhalf, q=4))
                    bo = dp.tile([NCORES, 128, 32 * nhalf], bf16, tag="bo",
                                 addr_space="Shared")
                    nc.gpsimd.collective_compute(
                        "AllGather", ALU.bypass, replica_groups=rg,
                        ins=[bi[:].opt()], outs=[bo[:].opt()])
                    bo_v = bo[:].rearrange("k p (l b) -> p k l b", l=nhalf)

                    if tau < T + 1:
                        # hx(tau+1) = [h0(tau) | h1(tau-2)]
                        hxn = hp.tile([128, KT * 2 * 32], bf16, tag="hx")
                        hxn_k = hxn[:].rearrange("p (k h b) -> p k h b",
                                                 k=KT, h=2)
                        if has_g0 and has_g1:
                            nc.sync.dma_start(
                                hxn[:].rearrange("p (k h b) -> p k h b",
                                                 k=KT, h=2),
                                bo_v[:])
                        elif has_g0:
                            nc.sync.dma_start(hxn_k[:, :, 0, :],
                                              bo_v[:, :, 0, :])
                            if tau == 1:
                                # g1(2) needs h1(-1) = enc
                                nc.sync.dma_start(hxn_k[:, :, 1, :],
                                                  ench_k[:])
                        else:
                            nc.sync.dma_start(hxn_k[:, :, 1, :],
                                              bo_v[:, :, 0, :])
                            # keep h0 half from previous hx (not used again)
                        hx_prev = hx
                        hx = hxn
                    else:
                        hx_prev = hx

                    # ---- h1 store; fp8 cast deferred 2 ticks so the DVE
                    # queue is not head-blocked waiting on the fresh AG ----
                    if has_g1:
                        t1_ = tau - 2
                        sl = slice(t1_ * 32, (t1_ + 1) * 32)
                        li = 1 if has_g0 else 0
                        nc.sync.dma_start(h1s_k[:, :, sl], bo_v[:, :, li, :])
                        cast_queue.append(sl)
                    if len(cast_queue) > 1:
                        slq = cast_queue.pop(0)
                        nc.vector.tensor_scalar_mul(
                            h8_k[:, :, slq], h1s_k[:, :, slq], SH)

                    # ---- AR reduces: after the AG so the gpsimd FIFO runs
                    # the AG first ----
                    if tau == 23:
                        emit_ar_reduce(0, 4)
                    elif tau == 39:
                        emit_ar_reduce(4, 8)

                    # ---- pre-pass interleave ----
                    if tau < NMT - 1:
                        emit_prepass_m(tau + 1)

                # tail: leftover casts + vocab pairs + ARs + outputs
                for slq in cast_queue:
                    nc.vector.tensor_scalar_mul(
                        h8_k[:, :, slq], h1s_k[:, :, slq], SH)
                cast_queue = []
                for m in range(NMT):
                    for j in range(4):
                        tick = 4 * m + 7 + j
                        if tick >= T + 2:
                            emit_vocab_pair(m, j)
                emit_ar_reduce(8, 13)
                emit_ar_out(8, 13)
                if debug_out:
                    nc.sync.dma_start(d_dbg_gxb[:], gxbd[:])
                    nc.sync.dma_start(d_dbg_h1s[:], h1store[:])

    nc.compile()
    return nc


def _get_nc(reps=1, use_linb=False, debug_out=False):
    key = ("nc", reps, use_linb, debug_out)
    if key not in _BUILD_CACHE:
        _BUILD_CACHE[key] = _build(reps, use_linb, debug_out)
    return _BUILD_CACHE[key]


def run(inputs, trace=False, reps=1, debug_out=False):
    from concourse.bass_utils import run_bass_kernel_spmd

    in_maps, use_linb = _host_prep(inputs)
    nc = _get_nc(reps, use_linb, debug_out)
    res = run_bass_kernel_spmd(nc, in_maps, core_ids=list(range(NCORES)),
                               trace=trace)
    full = np.empty((S, V), np.float32)
    for r in range(NCORES):
        full[:, r * VS:(r + 1) * VS] = res.results[r]["out"]
    return full, res


def kernel(**inputs):
    full, _ = run(inputs)
    return full
